# revision 6
# baseline (speedup 1.0000x reference)
"""Trainium2 Bass kernel for nn_MeshNN (piecewise-linear mesh interpolation).

The reference computes u(x) = sum_i w_uu[i]*phi_i(x) + w_dd.boundary(x) where
phi_i are clamped hat shape functions -> u is piecewise-linear in x. Host-side
we build (from the tiny coords/w_uu/w_dd inputs, O(NP) work) a per-bin table
over 5120 uniform bins of width 1/512 so that within bin j:

    u(x) = Q + M*(x - E) + D1*relu(x - B1) + D2*relu(x - B2)

bin index j = floor(x*512) (exact in f32 since 512 = 2^9). The device kernel
(data-parallel over 8 NeuronCores, 16384 points each) computes indices on the
DVE, gathers per-point table rows with dma_gather (256B rows), evaluates the
4-term PWL form, and writes u back.

Degenerate-node semantics (node with b == c, a 0/0 in the reference) follow
the neuron-backend behaviour relu(NaN) = 0, i.e. phi = top - 1.
"""
import numpy as np
import concourse.bacc as bacc
import concourse.bass as bass
from concourse import mybir
from concourse.bass_utils import run_bass_kernel_spmd
from concourse.library_config import mlp

F32 = mybir.dt.float32
I16 = mybir.dt.int16

N = 131072
NC_COUNT = 8
PTS = N // NC_COUNT            # 16384 points per core
P = 128                        # partitions
FD = PTS // P                  # 128 free dim for eval layout
WRAP = PTS // 16               # 1024 free dim for wrapped idx layout
NBINS = 5120                   # bins of width 1/512 over [0, 10)
SCALE = 512.0
ROW = 8                        # useful f32 per table row
ESIZE = 64                     # padded row: 64 f32 = 256B (dma_gather minimum)

_TP = np.float32(1.0 - 1.0 / 150.0)
_TM = np.float32(1.0 + 1.0 / 150.0)


# ---------------------------------------------------------------- host: table
def _node_params(coords, w_uu, w_dd):
    """Per-node (a, b, c, w) in f32, exactly as the reference computes them."""
    f = np.float32
    a = coords[:-2]
    xm = coords[1:-1]
    c = coords[2:]
    xi = np.minimum(xm, (_TP * c).astype(f))
    xi = np.maximum(xi, (_TM * a).astype(f))
    nodes = [(a[j], xi[j], c[j], w_uu[j]) for j in range(len(a))]
    a0 = f(coords[0] - f(coords[-1] / f(100.0)))
    b0 = np.maximum(np.minimum(coords[0], f(_TP * coords[1])), f(_TM * a0))
    nodes.append((a0, b0, coords[1], w_dd[0]))
    cL = f(coords[-1] * f(1.0 + 1.0 / 100.0))
    bL = np.maximum(np.minimum(coords[-1], f(_TP * cL)), f(_TM * coords[-2]))
    nodes.append((coords[-2], bL, cL, w_dd[1]))
    return nodes


def _build_table(coords, w_uu, w_dd):
    """[NBINS, ROW] f32 rows [M, Q, B1, D1, B2, D2, E, 0] built in f64."""
    # u(x) = K + sum_m d_m * relu(x - t_m)
    K = 0.0
    kinks = []
    for a, b, c, w in _node_params(coords, w_uu, w_dd):
        a = float(a); b = float(b); c = float(c); w = float(w)
        assert b > a, (a, b, c)
        s1 = 1.0 / (b - a)
        if c > b:            # normal hat
            s2 = 1.0 / (c - b)
            kinks += [(a, w * s1), (b, -w * (s1 + s2)), (c, w * s2)]
        elif c == b:         # degenerate: phi = top - 1 (neuron relu(NaN)=0)
            K += -w
            kinks += [(a, w * s1), (b, -w * s1)]
        else:                # clamped past c: rise then growing linear tail
            s3 = 1.0 / (b - c)
            kinks += [(a, w * s1), (b, w * (-s1 + s3))]
    kinks.sort()
    t = np.array([k[0] for k in kinks])
    d = np.array([k[1] for k in kinks])
    edges = np.arange(NBINS + 1) / SCALE

    idx_r = np.searchsorted(t, edges, side="right")
    idx_l = np.searchsorted(t, edges, side="left")
    csum_d = np.concatenate([[0.0], np.cumsum(d)])
    csum_dt = np.concatenate([[0.0], np.cumsum(d * t)])

    table = np.zeros((NBINS, ROW), dtype=np.float32)
    for j in range(NBINS):
        E = edges[j]
        n = idx_r[j]
        M = csum_d[n]
        Q = K + M * E - csum_dt[n]
        lo, hi = idx_r[j], idx_l[j + 1]       # interior kinks E_j < t < E_{j+1}
        ks = []
        i = lo
        while i < hi:
            tt, dd = t[i], d[i]
            while i + 1 < hi and t[i + 1] == tt:
                i += 1
                dd += d[i]
            ks.append((tt, dd))
            i += 1
        assert len(ks) <= 2, (j, ks)
        row = [M, Q]
        for tt, dd in ks:
            row += [tt, dd]
        while len(row) < 6:
            row += [E, 0.0]
        row += [E, 0.0]
        table[j] = np.array(row, dtype=np.float32)
    return table


# ------------------------------------------------------------- device program
_CACHED_NC = None


GCHUNK = 4096                  # dma_gather indices per instruction (ring limit)
NG = PTS // GCHUNK             # 4 gather instructions


def _build_nc():
    nc = bacc.Bacc("TRN2", debug=False, num_devices=NC_COUNT,
                   dynamic_dma_scratch_size=65536)

    x_eval = nc.dram_tensor("x_eval", [P, FD], F32, kind="ExternalInput")
    x_wrap = nc.dram_tensor("x_wrap", [P, WRAP], F32, kind="ExternalInput")
    tbl = nc.dram_tensor("tbl", [NBINS, ESIZE], F32, kind="ExternalInput")
    u_out = nc.dram_tensor("u_out", [P, FD], F32, kind="ExternalOutput")

    with (
        nc.sbuf_tensor("xw", [P, WRAP], F32) as xw,
        nc.sbuf_tensor("xe", [P, FD], F32) as xe,
        nc.sbuf_tensor("idx", [P, WRAP], I16) as idx,
        nc.sbuf_tensor("gt", [P, FD, ESIZE], F32) as gt,
        nc.sbuf_tensor("t0", [P, FD], F32) as t0,
        nc.sbuf_tensor("t1", [P, FD], F32) as t1,
        nc.sbuf_tensor("u", [P, FD], F32) as u,
        nc.semaphore("s_dma") as s_dma,
        nc.semaphore("s_g") as s_g,
        nc.semaphore("s_v") as s_v,
    ):
        M = gt[:, :, 0]
        Q = gt[:, :, 1]
        B1 = gt[:, :, 2]
        D1 = gt[:, :, 3]
        B2 = gt[:, :, 4]
        D2 = gt[:, :, 5]
        E = gt[:, :, 6]
        OP = mybir.AluOpType

        with nc.Block() as block:

            @block.gpsimd
            def _(g):
                g.load_library(mlp)
                g.dma_start(xw[:, :], x_wrap[:, :]).then_inc(s_dma, 16)
                g.dma_start(xe[:, :], x_eval[:, :]).then_inc(s_dma, 16)
                g.wait_ge(s_v, 1)
                for k in range(NG):
                    ii = k * (GCHUNK // P)
                    g.dma_gather(
                        gt[:, ii:ii + GCHUNK // P, :], tbl[:, :],
                        idx[:, k * (GCHUNK // 16):(k + 1) * (GCHUNK // 16)],
                        GCHUNK, GCHUNK, ESIZE, single_packet=False,
                    ).then_inc(s_g, 16)
                g.wait_ge(s_v, 2)
                g.dma_start(u_out[:, :], u[:, :]).then_inc(s_dma, 16)
                g.wait_ge(s_dma, 48)
                g.wait_ge(s_g, 16 * NG)

            @block.vector
            def _(v):
                v.wait_ge(s_dma, 16)
                # idx = rne(x*512 - 0.5) == floor(x*512) up to benign edge ties
                v.tensor_scalar(idx[:, :], xw[:, :], SCALE, 0.5,
                                op0=OP.mult, op1=OP.subtract).then_inc(s_v, 1)
                v.wait_ge(s_dma, 32)       # xe landed
                v.wait_ge(s_g, 16 * NG)    # all gathers landed
                # u = Q + M*(x-E) + D1*relu(x-B1) + D2*relu(x-B2)
                v.tensor_tensor(t0[:, :], xe[:, :], E, op=OP.subtract)
                v.tensor_tensor(t0[:, :], t0[:, :], M, op=OP.mult)
                v.tensor_tensor(u[:, :], t0[:, :], Q, op=OP.add)
                # relu(x-B) computed exactly as max(x,B) - B
                v.tensor_tensor(t1[:, :], xe[:, :], B1, op=OP.max)
                v.tensor_tensor(t1[:, :], t1[:, :], B1, op=OP.subtract)
                v.tensor_tensor(t1[:, :], t1[:, :], D1, op=OP.mult)
                v.tensor_tensor(u[:, :], u[:, :], t1[:, :], op=OP.add)
                v.tensor_tensor(t1[:, :], xe[:, :], B2, op=OP.max)
                v.tensor_tensor(t1[:, :], t1[:, :], B2, op=OP.subtract)
                v.tensor_tensor(t1[:, :], t1[:, :], D2, op=OP.mult)
                v.tensor_tensor(u[:, :], u[:, :], t1[:, :], op=OP.add).then_inc(s_v, 1)

    nc.compile()
    return nc


def kernel(x, coords, w_uu, w_dd):
    global _CACHED_NC
    x = np.asarray(x, np.float32)
    coords = np.asarray(coords, np.float32)
    w_uu = np.asarray(w_uu, np.float32)
    w_dd = np.asarray(w_dd, np.float32)

    table = _build_table(coords, w_uu, w_dd)
    tbl_pad = np.zeros((NBINS, ESIZE), np.float32)
    tbl_pad[:, :ROW] = table

    if _CACHED_NC is None:
        _CACHED_NC = _build_nc()
    nc = _CACHED_NC

    xf = x.ravel()
    in_maps = []
    for c in range(NC_COUNT):
        shard = xf[c * PTS:(c + 1) * PTS]
        x_eval = np.ascontiguousarray(shard.reshape(FD, P).T)       # [p, i] = shard[i*P+p]
        x_wrap16 = shard.reshape(WRAP, 16).T                        # [p16, j] = shard[j*16+p16]
        x_wrap = np.ascontiguousarray(np.tile(x_wrap16, (8, 1)))    # replicate to 8 Q7 groups
        in_maps.append({"x_eval": x_eval, "x_wrap": x_wrap, "tbl": tbl_pad})

    res = run_bass_kernel_spmd(nc, in_maps, core_ids=list(range(NC_COUNT)))

    out = np.empty(N, np.float32)
    for c in range(NC_COUNT):
        u = res.results[c]["u_out"]                                 # [p, i]
        out[c * PTS:(c + 1) * PTS] = u.T.ravel()
    return out.reshape(1, N, 1)


# revision 8
# speedup vs baseline: 1.0644x; 1.0644x over previous
"""Trainium2 Bass kernel for nn_MeshNN (piecewise-linear mesh interpolation).

The reference computes u(x) = sum_i w_uu[i]*phi_i(x) + w_dd.boundary(x) where
phi_i are clamped hat shape functions -> u is piecewise-linear in x. Host-side
we build (from the tiny coords/w_uu/w_dd inputs, O(NP) work) a per-bin table
over 5120 uniform bins of width 1/512 so that within bin j:

    u(x) = Q + M*(x - E) + D1*relu(x - B1) + D2*relu(x - B2)

bin index j = floor(x*512) (exact in f32 since 512 = 2^9). The device kernel
(data-parallel over 8 NeuronCores, 16384 points each) computes indices on the
DVE, gathers per-point table rows with dma_gather (256B rows), evaluates the
4-term PWL form, and writes u back.

Degenerate-node semantics (node with b == c, a 0/0 in the reference) follow
the neuron-backend behaviour relu(NaN) = 0, i.e. phi = top - 1.
"""
import numpy as np
import concourse.bacc as bacc
import concourse.bass as bass
from concourse import mybir
from concourse.bass_utils import run_bass_kernel_spmd
from concourse.library_config import mlp

F32 = mybir.dt.float32
I16 = mybir.dt.int16

N = 131072
NC_COUNT = 8
PTS = N // NC_COUNT            # 16384 points per core
P = 128                        # partitions
FD = PTS // P                  # 128 free dim for eval layout
WRAP = PTS // 16               # 1024 free dim for wrapped idx layout
NBINS = 5120                   # bins of width 1/512 over [0, 10)
SCALE = 512.0
ROW = 8                        # useful f32 per table row
ESIZE = 64                     # padded row: 64 f32 = 256B (dma_gather minimum)

_TP = np.float32(1.0 - 1.0 / 150.0)
_TM = np.float32(1.0 + 1.0 / 150.0)


# ---------------------------------------------------------------- host: table
def _node_params(coords, w_uu, w_dd):
    """Per-node (a, b, c, w) in f32, exactly as the reference computes them."""
    f = np.float32
    a = coords[:-2]
    xm = coords[1:-1]
    c = coords[2:]
    xi = np.minimum(xm, (_TP * c).astype(f))
    xi = np.maximum(xi, (_TM * a).astype(f))
    nodes = [(a[j], xi[j], c[j], w_uu[j]) for j in range(len(a))]
    a0 = f(coords[0] - f(coords[-1] / f(100.0)))
    b0 = np.maximum(np.minimum(coords[0], f(_TP * coords[1])), f(_TM * a0))
    nodes.append((a0, b0, coords[1], w_dd[0]))
    cL = f(coords[-1] * f(1.0 + 1.0 / 100.0))
    bL = np.maximum(np.minimum(coords[-1], f(_TP * cL)), f(_TM * coords[-2]))
    nodes.append((coords[-2], bL, cL, w_dd[1]))
    return nodes


def _build_table(coords, w_uu, w_dd):
    """[NBINS, ROW] f32 rows [M, Q, B1, D1, B2, D2, E, 0] built in f64."""
    # u(x) = K + sum_m d_m * relu(x - t_m)
    K = 0.0
    kinks = []
    for a, b, c, w in _node_params(coords, w_uu, w_dd):
        a = float(a); b = float(b); c = float(c); w = float(w)
        assert b > a, (a, b, c)
        s1 = 1.0 / (b - a)
        if c > b:            # normal hat
            s2 = 1.0 / (c - b)
            kinks += [(a, w * s1), (b, -w * (s1 + s2)), (c, w * s2)]
        elif c == b:         # degenerate: phi = top - 1 (neuron relu(NaN)=0)
            K += -w
            kinks += [(a, w * s1), (b, -w * s1)]
        else:                # clamped past c: rise then growing linear tail
            s3 = 1.0 / (b - c)
            kinks += [(a, w * s1), (b, w * (-s1 + s3))]
    kinks.sort()
    t = np.array([k[0] for k in kinks])
    d = np.array([k[1] for k in kinks])
    edges = np.arange(NBINS + 1) / SCALE

    idx_r = np.searchsorted(t, edges, side="right")
    idx_l = np.searchsorted(t, edges, side="left")
    csum_d = np.concatenate([[0.0], np.cumsum(d)])
    csum_dt = np.concatenate([[0.0], np.cumsum(d * t)])

    table = np.zeros((NBINS, ROW), dtype=np.float32)
    for j in range(NBINS):
        E = edges[j]
        n = idx_r[j]
        M = csum_d[n]
        Q = K + M * E - csum_dt[n]
        lo, hi = idx_r[j], idx_l[j + 1]       # interior kinks E_j < t < E_{j+1}
        ks = []
        i = lo
        while i < hi:
            tt, dd = t[i], d[i]
            while i + 1 < hi and t[i + 1] == tt:
                i += 1
                dd += d[i]
            ks.append((tt, dd))
            i += 1
        assert len(ks) <= 2, (j, ks)
        row = [M, Q]
        for tt, dd in ks:
            row += [tt, dd]
        while len(row) < 6:
            row += [E, 0.0]
        row += [E, 0.0]
        table[j] = np.array(row, dtype=np.float32)
    return table


# ------------------------------------------------------------- device program
_CACHED_NC = None


GCHUNK = 4096                  # dma_gather indices per instruction (ring limit)
NG = PTS // GCHUNK             # 4 gather instructions


def _build_nc():
    nc = bacc.Bacc("TRN2", debug=False, num_devices=NC_COUNT,
                   dynamic_dma_scratch_size=65536)

    x_eval = nc.dram_tensor("x_eval", [P, FD], F32, kind="ExternalInput")
    x_wrap = nc.dram_tensor("x_wrap", [P, WRAP], F32, kind="ExternalInput")
    tbl = nc.dram_tensor("tbl", [NBINS, ESIZE], F32, kind="ExternalInput")
    u_out = nc.dram_tensor("u_out", [P, FD], F32, kind="ExternalOutput")

    with (
        nc.sbuf_tensor("xw", [P, WRAP], F32) as xw,
        nc.sbuf_tensor("xe", [P, FD], F32) as xe,
        nc.sbuf_tensor("idx", [P, WRAP], I16) as idx,
        nc.sbuf_tensor("gt", [P, FD, ESIZE], F32) as gt,
        nc.sbuf_tensor("t0", [P, FD], F32) as t0,
        nc.sbuf_tensor("t1", [P, FD], F32) as t1,
        nc.sbuf_tensor("u", [P, FD], F32) as u,
        nc.semaphore("s_in") as s_in,
        nc.semaphore("s_xe") as s_xe,
        nc.semaphore("s_out") as s_out,
        nc.semaphore("s_g") as s_g,
        nc.semaphore("s_v") as s_v,
    ):
        M = gt[:, :, 0]
        Q = gt[:, :, 1]
        B1 = gt[:, :, 2]
        D1 = gt[:, :, 3]
        B2 = gt[:, :, 4]
        D2 = gt[:, :, 5]
        E = gt[:, :, 6]
        OP = mybir.AluOpType

        HW = WRAP // 2           # wrapped-layout columns per half (512)
        HF = FD // 2             # eval-layout columns per half (64)
        HC = NG // 2             # gather chunks per half

        with nc.Block() as block:

            @block.gpsimd
            def _(g):
                g.load_library(mlp)
                g.dma_start(xw[:, :HW], x_wrap[:, :HW]).then_inc(s_in, 16)
                g.dma_start(xw[:, HW:], x_wrap[:, HW:]).then_inc(s_in, 16)
                for k in range(NG):
                    if k % HC == 0:
                        g.wait_ge(s_v, k // HC + 1)
                    ii = k * (GCHUNK // P)
                    g.dma_gather(
                        gt[:, ii:ii + GCHUNK // P, :], tbl[:, :],
                        idx[:, k * (GCHUNK // 16):(k + 1) * (GCHUNK // 16)],
                        GCHUNK, GCHUNK, ESIZE, single_packet=False,
                    ).then_inc(s_g, 16)
                g.wait_ge(s_g, 16 * NG)

            @block.sync
            def _(s):
                s.dma_start(xe[:, :], x_eval[:, :]).then_inc(s_xe, 16)
                s.wait_ge(s_v, 3)
                s.dma_start(u_out[:, :HF], u[:, :HF]).then_inc(s_out, 16)
                s.wait_ge(s_v, 4)
                s.dma_start(u_out[:, HF:], u[:, HF:]).then_inc(s_out, 16)
                s.wait_ge(s_out, 32)

            @block.vector
            def _(v):
                # idx = rne(x*512 - 0.5) == floor(x*512) up to benign edge ties
                v.wait_ge(s_in, 16)
                v.tensor_scalar(idx[:, :HW], xw[:, :HW], SCALE, 0.5,
                                op0=OP.mult, op1=OP.subtract).then_inc(s_v, 1)
                v.wait_ge(s_in, 32)
                v.tensor_scalar(idx[:, HW:], xw[:, HW:], SCALE, 0.5,
                                op0=OP.mult, op1=OP.subtract).then_inc(s_v, 1)
                v.wait_ge(s_xe, 16)
                # u = Q + M*(x-E) + D1*relu(x-B1) + D2*relu(x-B2), per half,
                # each half evaluated as soon as its gather chunks land.
                for h in range(2):
                    v.wait_ge(s_g, 16 * HC * (h + 1))
                    c = slice(h * HF, (h + 1) * HF)
                    xeh = xe[:, c]
                    t0h, t1h, uh = t0[:, c], t1[:, c], u[:, c]
                    Mh, Qh, Eh = M[:, c], Q[:, c], E[:, c]
                    B1h, D1h = B1[:, c], D1[:, c]
                    B2h, D2h = B2[:, c], D2[:, c]
                    v.tensor_tensor(t0h, xeh, Eh, op=OP.subtract)
                    v.tensor_tensor(t0h, t0h, Mh, op=OP.mult)
                    v.tensor_tensor(uh, t0h, Qh, op=OP.add)
                    # relu(x-B) computed exactly as max(x,B) - B
                    v.tensor_tensor(t1h, xeh, B1h, op=OP.max)
                    v.tensor_tensor(t1h, t1h, B1h, op=OP.subtract)
                    v.tensor_tensor(t1h, t1h, D1h, op=OP.mult)
                    v.tensor_tensor(uh, uh, t1h, op=OP.add)
                    v.tensor_tensor(t1h, xeh, B2h, op=OP.max)
                    v.tensor_tensor(t1h, t1h, B2h, op=OP.subtract)
                    v.tensor_tensor(t1h, t1h, D2h, op=OP.mult)
                    v.tensor_tensor(uh, uh, t1h, op=OP.add).then_inc(s_v, 1)

    nc.compile()
    return nc


def kernel(x, coords, w_uu, w_dd):
    global _CACHED_NC
    x = np.asarray(x, np.float32)
    coords = np.asarray(coords, np.float32)
    w_uu = np.asarray(w_uu, np.float32)
    w_dd = np.asarray(w_dd, np.float32)

    table = _build_table(coords, w_uu, w_dd)
    tbl_pad = np.zeros((NBINS, ESIZE), np.float32)
    tbl_pad[:, :ROW] = table

    if _CACHED_NC is None:
        _CACHED_NC = _build_nc()
    nc = _CACHED_NC

    xf = x.ravel()
    in_maps = []
    for c in range(NC_COUNT):
        shard = xf[c * PTS:(c + 1) * PTS]
        x_eval = np.ascontiguousarray(shard.reshape(FD, P).T)       # [p, i] = shard[i*P+p]
        x_wrap16 = shard.reshape(WRAP, 16).T                        # [p16, j] = shard[j*16+p16]
        x_wrap = np.ascontiguousarray(np.tile(x_wrap16, (8, 1)))    # replicate to 8 Q7 groups
        in_maps.append({"x_eval": x_eval, "x_wrap": x_wrap, "tbl": tbl_pad})

    res = run_bass_kernel_spmd(nc, in_maps, core_ids=list(range(NC_COUNT)))

    out = np.empty(N, np.float32)
    for c in range(NC_COUNT):
        u = res.results[c]["u_out"]                                 # [p, i]
        out[c * PTS:(c + 1) * PTS] = u.T.ravel()
    return out.reshape(1, N, 1)
